# revision 6
# baseline (speedup 1.0000x reference)
"""MoE (top-1 routing, E=8) Trainium2 Bass kernel.

Full-input contract: kernel(**inputs) takes the unsharded numpy inputs of
reference.setup_inputs() and returns the full [N, H] float32 output.

Strategy (token-parallel SPMD over 8 NeuronCores, 2048 tokens/core):
  host:   fp32 router (x @ Wr.T + br, argmax); per-core expert-sorted
          "capacity layout" (per-slot 128-aligned segments, experts
          permuted per core by descending count so one static capacity
          profile fits every core); tokens pre-gathered into the sorted
          layout; fp16 casts; weights pre-transposed so the contraction
          dim D lands on SBUF partitions.
  device: shared FFN = dense fp16 matmuls over token-order tiles
          (tokens stationary, H moving in 512-slices, K=8x128 chunks
          accumulated in PSUM fp32) + bs -> fp16 "out" rows.
          routed FFN = fp16 matmuls over the sorted capacity tiles with
          each slot's expert weights + be -> fp16 "routed" rows (dense,
          sorted layout).
  host:   out[token] = shared[token] + routed[slot_of(token)], fp32.
"""

import sys

sys.path.insert(0, "/opt/trn_rl_repo")

from dataclasses import dataclass

import numpy as np

import concourse.bass as bass
import concourse.mybir as mybir
from concourse.tile import TileContext

# ----------------------------------------------------------------------------
# configuration
# ----------------------------------------------------------------------------


@dataclass
class Cfg:
    n_loc: int = 2048          # tokens per core
    d: int = 1024              # input dim (contraction)
    h: int = 4096              # hidden dim
    e: int = 8                 # experts
    cap: tuple = (3, 3, 3, 3, 3, 2, 2, 2)  # tiles per sorted slot
    n_cores: int = 8

    @property
    def kc(self):
        return self.d // 128

    @property
    def ns(self):
        return self.h // 512

    @property
    def nt(self):
        return self.n_loc // 128

    @property
    def rt(self):
        return sum(self.cap)


F16 = mybir.dt.float16
F32 = mybir.dt.float32

MAX_WAITS = 1


def split_long_waits(nc, max_w: int = MAX_WAITS):
    """walrus TPB_CTRL codegen rejects instructions with multiple sync waits
    (CoreV3GenImpl setupSyncWait).  Tile's exit drain can exceed that; move
    excess waits onto same-engine NoOps inserted just before the offender."""
    n_fix = 0
    for f in nc.m.functions:
        for bb in f.blocks:
            insts = bb.instructions
            new_list = []
            changed = False
            for inst in insts:
                si = inst.sync_info
                if si is not None and len(si.on_wait) > max_w:
                    w = list(si.on_wait)
                    k = 0
                    while len(w) > max_w:
                        chunk, w = w[:max_w], w[max_w:]
                        nop = mybir.InstNoOp(
                            name=f"{inst.name}_waitsplit_{k}",
                            engine=inst.engine,
                            sync_info=mybir.SyncInfo(on_wait=chunk, on_update=[]),
                            bass_nofuse=True,
                        )
                        new_list.append(nop)
                        k += 1
                    inst.sync_info = mybir.SyncInfo(
                        on_wait=w, on_update=list(si.on_update)
                    )
                    n_fix += 1
                    changed = True
                new_list.append(inst)
            if changed:
                bb.instructions = new_list
    return n_fix


# ----------------------------------------------------------------------------
# device program
# ----------------------------------------------------------------------------


def build_program(cfg: Cfg, fix_waits: bool = True):
    nc = bass.Bass()

    xt = nc.declare_dram_parameter("xt16", [cfg.d, cfg.n_loc], F16, isOutput=False)
    xg_d = nc.declare_dram_parameter(
        "xg16", [128, cfg.kc * cfg.rt * 128], F16, isOutput=False
    )
    wst = nc.declare_dram_parameter("wst16", [cfg.d, cfg.h], F16, isOutput=False)
    wet = nc.declare_dram_parameter(
        "wet16", [cfg.e, cfg.d, cfg.h], F16, isOutput=False
    )
    bsr = nc.declare_dram_parameter("bs_rep", [128, cfg.h], F32, isOutput=False)
    ber = nc.declare_dram_parameter(
        "be_rep", [cfg.e, 128, cfg.h], F32, isOutput=False
    )
    outp = nc.declare_dram_parameter("out", [cfg.n_loc, cfg.h], F16, isOutput=True)
    routp = nc.declare_dram_parameter(
        "routed", [cfg.rt * 128, cfg.h], F16, isOutput=True
    )

    base = np.cumsum([0] + list(cfg.cap))  # slot -> first tile index
    cap_max = max(cfg.cap)

    with TileContext(nc) as tc:
        with (
            tc.tile_pool(name="resident", bufs=1) as rpool,
            tc.tile_pool(name="wsh", bufs=3) as wpool,
            tc.tile_pool(name="wrt", bufs=2) as wepool,
            tc.tile_pool(name="oshared", bufs=4) as opool,
            tc.tile_pool(name="stage", bufs=2) as stpool,
            tc.tile_pool(name="bias_e", bufs=1) as bpool,
            tc.tile_pool(name="ps", bufs=4, space="PSUM") as pspool,
        ):
            # ---- resident loads -------------------------------------------
            xts = rpool.tile([128, cfg.kc, cfg.n_loc], F16, tag="xts")
            for k in range(cfg.kc):
                nc.sync.dma_start(
                    out=xts[:, k, :], in_=xt[k * 128 : (k + 1) * 128, :]
                )

            xg = rpool.tile([128, cfg.kc, cfg.rt * 128], F16, tag="xg")
            nc.sync.dma_start(out=xg[:, :, :], in_=xg_d[:, :])

            bs_sb = rpool.tile([128, cfg.h], F32, tag="bs")
            nc.sync.dma_start(out=bs_sb[:, :], in_=bsr[:, :])

            # ---- shared FFN: dense token-order tiles ----------------------
            for n in range(cfg.ns):
                wtile = wpool.tile([128, cfg.kc, 512], F16, tag="ws")
                for k in range(cfg.kc):
                    nc.sync.dma_start(
                        out=wtile[:, k, :],
                        in_=wst[k * 128 : (k + 1) * 128, n * 512 : (n + 1) * 512],
                    )
                for t in range(cfg.nt):
                    ps = pspool.tile([128, 512], F32, tag="ps")
                    for k in range(cfg.kc):
                        nc.tensor.matmul(
                            ps[:, :],
                            lhsT=xts[:, k, t * 128 : (t + 1) * 128],
                            rhs=wtile[:, k, :],
                            start=(k == 0),
                            stop=(k == cfg.kc - 1),
                        )
                    ot = opool.tile([128, 512], F16, tag="osh")
                    nc.vector.tensor_add(
                        out=ot[:, :],
                        in0=ps[:, :],
                        in1=bs_sb[:, n * 512 : (n + 1) * 512],
                    )
                    nc.sync.dma_start(
                        out=outp[
                            t * 128 : (t + 1) * 128, n * 512 : (n + 1) * 512
                        ],
                        in_=ot[:, :],
                    )

            # ---- routed FFN: sorted capacity layout -----------------------
            for s in range(cfg.e):
                bes = bpool.tile([128, cfg.h], F32, tag="be")
                nc.sync.dma_start(out=bes[:, :], in_=ber[s, :, :])
                st = stpool.tile([128, cap_max, cfg.h], F16, tag="st")
                for n in range(cfg.ns):
                    wtile = wepool.tile([128, cfg.kc, 512], F16, tag="we")
                    for k in range(cfg.kc):
                        nc.sync.dma_start(
                            out=wtile[:, k, :],
                            in_=wet[
                                s,
                                k * 128 : (k + 1) * 128,
                                n * 512 : (n + 1) * 512,
                            ],
                        )
                    for tl in range(cfg.cap[s]):
                        t = base[s] + tl
                        ps = pspool.tile([128, 512], F32, tag="ps")
                        for k in range(cfg.kc):
                            nc.tensor.matmul(
                                ps[:, :],
                                lhsT=xg[:, k, t * 128 : (t + 1) * 128],
                                rhs=wtile[:, k, :],
                                start=(k == 0),
                                stop=(k == cfg.kc - 1),
                            )
                        nc.vector.tensor_add(
                            out=st[:, tl, n * 512 : (n + 1) * 512],
                            in0=ps[:, :],
                            in1=bes[:, n * 512 : (n + 1) * 512],
                        )
                for tl in range(cfg.cap[s]):
                    t = base[s] + tl
                    nc.sync.dma_start(
                        out=routp[t * 128 : (t + 1) * 128, :],
                        in_=st[:, tl, :],
                    )

    if fix_waits:
        split_long_waits(nc)
    return nc


# ----------------------------------------------------------------------------
# host-side routing / input prep / combine
# ----------------------------------------------------------------------------


def route_and_pack(cfg: Cfg, te):
    """Per-core routing tables.  te [n_loc] expert ids.

    Returns (perm, sorted_tokens, valid_mask): sorted_tokens [rt*128] maps
    capacity slot -> token id (pad slots -> token 0, valid_mask False)."""
    counts = np.bincount(te, minlength=cfg.e)
    perm = np.argsort(-counts, kind="stable")
    base = np.cumsum([0] + list(cfg.cap))
    sorted_tokens = np.zeros(cfg.rt * 128, dtype=np.int64)
    valid = np.zeros(cfg.rt * 128, dtype=bool)
    for s in range(cfg.e):
        ex = perm[s]
        toks = np.nonzero(te == ex)[0]
        assert len(toks) <= cfg.cap[s] * 128, (
            f"slot {s} expert {ex}: {len(toks)} tokens > capacity "
            f"{cfg.cap[s] * 128}"
        )
        off = base[s] * 128
        sorted_tokens[off : off + len(toks)] = toks
        valid[off : off + len(toks)] = True
    return perm, sorted_tokens, valid


def make_in_map(cfg: Cfg, xs, te, Ws, bs, We, be):
    perm, sorted_tokens, valid = route_and_pack(cfg, te)
    x16 = np.ascontiguousarray(xs).astype(np.float16)
    # gathered + transposed tokens in the [128, kc, rt*128] matmul layout
    xg = (
        x16[sorted_tokens]                       # [rt*128, d]
        .T.reshape(cfg.kc, 128, cfg.rt * 128)    # [kc, 128(part), rt*128]
        .transpose(1, 0, 2)                      # [128, kc, rt*128]
    )
    return {
        "xt16": np.ascontiguousarray(xs.T).astype(np.float16),
        "xg16": np.ascontiguousarray(xg).reshape(128, -1),
        "wst16": np.ascontiguousarray(Ws.T).astype(np.float16),
        "wet16": np.ascontiguousarray(
            We[perm].transpose(0, 2, 1)
        ).astype(np.float16),
        "bs_rep": np.ascontiguousarray(
            np.broadcast_to(bs.astype(np.float32), (128, cfg.h))
        ),
        "be_rep": np.ascontiguousarray(
            np.broadcast_to(
                be[perm].astype(np.float32)[:, None, :], (cfg.e, 128, cfg.h)
            )
        ),
    }, (sorted_tokens, valid)


def combine(cfg: Cfg, shared_out, routed_out, sorted_tokens, valid):
    out = shared_out.astype(np.float32)
    out[sorted_tokens[valid]] += routed_out[valid].astype(np.float32)
    return out


# ----------------------------------------------------------------------------
# entry point
# ----------------------------------------------------------------------------

_PROGRAM_CACHE = {}


def _get_program(cfg: Cfg):
    key = (cfg.n_loc, cfg.d, cfg.h, cfg.e, cfg.cap)
    if key not in _PROGRAM_CACHE:
        _PROGRAM_CACHE[key] = build_program(cfg)
    return _PROGRAM_CACHE[key]


def kernel(x, Ws, bs, We, be, Wr, br):
    from concourse.bass_utils import run_bass_kernel_spmd

    cfg = Cfg()
    x = np.asarray(x, dtype=np.float32)
    Ws = np.asarray(Ws, dtype=np.float32)
    bs = np.asarray(bs, dtype=np.float32)
    We = np.asarray(We, dtype=np.float32)
    be = np.asarray(be, dtype=np.float32)
    Wr = np.asarray(Wr, dtype=np.float32)
    br = np.asarray(br, dtype=np.float32)

    n = x.shape[0]
    assert n == cfg.n_loc * cfg.n_cores

    logits = x @ Wr.T + br
    te = np.argmax(logits, axis=-1)

    nc = _get_program(cfg)
    in_maps, metas = [], []
    for c in range(cfg.n_cores):
        sl = slice(c * cfg.n_loc, (c + 1) * cfg.n_loc)
        m, meta = make_in_map(cfg, x[sl], te[sl], Ws, bs, We, be)
        in_maps.append(m)
        metas.append(meta)

    res = run_bass_kernel_spmd(nc, in_maps, list(range(cfg.n_cores)))
    outs = []
    for c in range(cfg.n_cores):
        st, valid = metas[c]
        outs.append(
            combine(cfg, res.results[c]["out"], res.results[c]["routed"], st, valid)
        )
    return np.concatenate(outs, axis=0)


# revision 8
# speedup vs baseline: 1.1317x; 1.1317x over previous
"""MoE (top-1 routing, E=8) Trainium2 Bass kernel.

Full-input contract: kernel(**inputs) takes the unsharded numpy inputs of
reference.setup_inputs() and returns the full [N, H] float32 output.

Strategy (token-parallel SPMD over 8 NeuronCores, 2048 tokens/core):
  host:   fp32 router (x @ Wr.T + br, argmax); per-core expert-sorted
          "capacity layout" (per-slot 128-aligned segments, experts
          permuted per core by descending count so one static capacity
          profile fits every core); tokens pre-gathered into the sorted
          layout; fp16 casts; weights/activations pre-tiled so every
          device DMA is one large fully-contiguous transfer with the
          contraction dim D on SBUF partitions.
  device: shared FFN = dense fp16 matmuls over token-order tiles
          (tokens stationary, H moving in 512-slices, K=8x128 chunks
          accumulated in PSUM fp32) + bs -> fp16 "out" rows.
          routed FFN = fp16 matmuls over the sorted capacity tiles with
          each slot's expert weights + be -> fp16 "routed" rows (dense,
          sorted layout).
  host:   out[token] = shared[token] + routed[slot_of(token)], fp32.
"""

import sys

sys.path.insert(0, "/opt/trn_rl_repo")

from dataclasses import dataclass

import numpy as np

import concourse.bass as bass
import concourse.mybir as mybir
from concourse.tile import TileContext

# ----------------------------------------------------------------------------
# configuration
# ----------------------------------------------------------------------------


@dataclass
class Cfg:
    n_loc: int = 2048          # tokens per core
    d: int = 1024              # input dim (contraction)
    h: int = 4096              # hidden dim
    e: int = 8                 # experts
    cap: tuple = (3, 3, 3, 3, 3, 2, 2, 2)  # tiles per sorted slot
    n_cores: int = 8

    @property
    def kc(self):
        return self.d // 128

    @property
    def ns(self):
        return self.h // 512

    @property
    def nt(self):
        return self.n_loc // 128

    @property
    def rt(self):
        return sum(self.cap)


F16 = mybir.dt.float16
F32 = mybir.dt.float32

MAX_WAITS = 1


def split_long_waits(nc, max_w: int = MAX_WAITS):
    """walrus TPB_CTRL codegen rejects instructions with multiple sync waits
    (CoreV3GenImpl setupSyncWait).  Tile's exit drain can exceed that; move
    excess waits onto same-engine NoOps inserted just before the offender."""
    n_fix = 0
    for f in nc.m.functions:
        for bb in f.blocks:
            insts = bb.instructions
            new_list = []
            changed = False
            for inst in insts:
                si = inst.sync_info
                if si is not None and len(si.on_wait) > max_w:
                    w = list(si.on_wait)
                    k = 0
                    while len(w) > max_w:
                        chunk, w = w[:max_w], w[max_w:]
                        nop = mybir.InstNoOp(
                            name=f"{inst.name}_waitsplit_{k}",
                            engine=inst.engine,
                            sync_info=mybir.SyncInfo(on_wait=chunk, on_update=[]),
                            bass_nofuse=True,
                        )
                        new_list.append(nop)
                        k += 1
                    inst.sync_info = mybir.SyncInfo(
                        on_wait=w, on_update=list(si.on_update)
                    )
                    n_fix += 1
                    changed = True
                new_list.append(inst)
            if changed:
                bb.instructions = new_list
    return n_fix


# ----------------------------------------------------------------------------
# device program
# ----------------------------------------------------------------------------


def build_program(cfg: Cfg, fix_waits: bool = True):
    nc = bass.Bass()

    # all activation/weight params are pre-tiled on the host so that each
    # DMA below is a single fully-contiguous transfer.
    xt = nc.declare_dram_parameter(
        "xt16", [128, cfg.kc * cfg.n_loc], F16, isOutput=False
    )
    xg_d = nc.declare_dram_parameter(
        "xg16", [128, cfg.kc * cfg.rt * 128], F16, isOutput=False
    )
    wst = nc.declare_dram_parameter(
        "wst16", [128, cfg.kc * cfg.h], F16, isOutput=False
    )
    wet = nc.declare_dram_parameter(
        "wet16", [cfg.e * cfg.ns, 128, cfg.kc * 512], F16, isOutput=False
    )
    bsr = nc.declare_dram_parameter("bs_rep", [128, cfg.h], F32, isOutput=False)
    ber = nc.declare_dram_parameter(
        "be_rep", [cfg.e, 128, cfg.h], F32, isOutput=False
    )
    outp = nc.declare_dram_parameter("out", [cfg.n_loc, cfg.h], F16, isOutput=True)
    routp = nc.declare_dram_parameter(
        "routed", [cfg.rt * 128, cfg.h], F16, isOutput=True
    )

    base = np.cumsum([0] + list(cfg.cap))  # slot -> first tile index
    cap_max = max(cfg.cap)

    with TileContext(nc) as tc:
        with (
            tc.tile_pool(name="resident", bufs=1) as rpool,
            tc.tile_pool(name="ps", bufs=4, space="PSUM") as pspool,
        ):
            # ---- resident loads (each one contiguous DMA) -----------------
            xg = rpool.tile([128, cfg.kc, cfg.rt * 128], F16, tag="xg")
            nc.sync.dma_start(out=xg[:, :, :], in_=xg_d[:, :])

            # ---- shared FFN: dense token-order tiles, full-row stores -----
            with (
                tc.tile_pool(name="sh_res", bufs=1) as spool,
                tc.tile_pool(name="oshared", bufs=3) as opool,
            ):
                xts = spool.tile([128, cfg.kc, cfg.n_loc], F16, tag="xts")
                nc.sync.dma_start(out=xts[:, :, :], in_=xt[:, :])
                wsall = spool.tile([128, cfg.kc, cfg.h], F16, tag="ws")
                nc.sync.dma_start(out=wsall[:, :, :], in_=wst[:, :])
                bs_sb = spool.tile([128, cfg.h], F32, tag="bs")
                nc.sync.dma_start(out=bs_sb[:, :], in_=bsr[:, :])

                for t in range(cfg.nt):
                    sh = opool.tile([128, cfg.h], F16, tag="osh")
                    for n in range(cfg.ns):
                        ps = pspool.tile([128, 512], F32, tag="ps")
                        for k in range(cfg.kc):
                            nc.tensor.matmul(
                                ps[:, :],
                                lhsT=xts[:, k, t * 128 : (t + 1) * 128],
                                rhs=wsall[:, k, n * 512 : (n + 1) * 512],
                                start=(k == 0),
                                stop=(k == cfg.kc - 1),
                            )
                        nc.vector.tensor_add(
                            out=sh[:, n * 512 : (n + 1) * 512],
                            in0=ps[:, :],
                            in1=bs_sb[:, n * 512 : (n + 1) * 512],
                        )
                    nc.sync.dma_start(
                        out=outp[t * 128 : (t + 1) * 128, :], in_=sh[:, :]
                    )

            # ---- routed FFN: sorted capacity layout -----------------------
            with (
                tc.tile_pool(name="wrt", bufs=3) as wepool,
                tc.tile_pool(name="stage", bufs=2) as stpool,
                tc.tile_pool(name="bias_e", bufs=2) as bpool,
            ):
                for s in range(cfg.e):
                    bes = bpool.tile([128, cfg.h], F32, tag="be")
                    nc.sync.dma_start(out=bes[:, :], in_=ber[s, :, :])
                    st = stpool.tile([128, cap_max, cfg.h], F16, tag="st")
                    for n in range(cfg.ns):
                        wtile = wepool.tile([128, cfg.kc, 512], F16, tag="we")
                        nc.sync.dma_start(
                            out=wtile[:, :, :], in_=wet[s * cfg.ns + n, :, :]
                        )
                        for tl in range(cfg.cap[s]):
                            t = base[s] + tl
                            ps = pspool.tile([128, 512], F32, tag="ps")
                            for k in range(cfg.kc):
                                nc.tensor.matmul(
                                    ps[:, :],
                                    lhsT=xg[:, k, t * 128 : (t + 1) * 128],
                                    rhs=wtile[:, k, :],
                                    start=(k == 0),
                                    stop=(k == cfg.kc - 1),
                                )
                            nc.vector.tensor_add(
                                out=st[:, tl, n * 512 : (n + 1) * 512],
                                in0=ps[:, :],
                                in1=bes[:, n * 512 : (n + 1) * 512],
                            )
                    for tl in range(cfg.cap[s]):
                        t = base[s] + tl
                        nc.sync.dma_start(
                            out=routp[t * 128 : (t + 1) * 128, :],
                            in_=st[:, tl, :],
                        )

    if fix_waits:
        split_long_waits(nc)
    return nc


# ----------------------------------------------------------------------------
# host-side routing / input prep / combine
# ----------------------------------------------------------------------------


def _part_tile(a, kc):
    """[kc*128, F] -> [128, kc*F] with [p, k*F+j] = a[k*128+p, j]."""
    f = a.shape[1]
    return a.reshape(kc, 128, f).transpose(1, 0, 2).reshape(128, kc * f)


def route_and_pack(cfg: Cfg, te):
    """Per-core routing tables.  te [n_loc] expert ids.

    Returns (perm, sorted_tokens, valid): sorted_tokens [rt*128] maps
    capacity slot -> token id (pad slots -> token 0, valid False)."""
    counts = np.bincount(te, minlength=cfg.e)
    perm = np.argsort(-counts, kind="stable")
    base = np.cumsum([0] + list(cfg.cap))
    sorted_tokens = np.zeros(cfg.rt * 128, dtype=np.int64)
    valid = np.zeros(cfg.rt * 128, dtype=bool)
    for s in range(cfg.e):
        ex = perm[s]
        toks = np.nonzero(te == ex)[0]
        assert len(toks) <= cfg.cap[s] * 128, (
            f"slot {s} expert {ex}: {len(toks)} tokens > capacity "
            f"{cfg.cap[s] * 128}"
        )
        off = base[s] * 128
        sorted_tokens[off : off + len(toks)] = toks
        valid[off : off + len(toks)] = True
    return perm, sorted_tokens, valid


def make_in_map(cfg: Cfg, xs, te, Ws, bs, We, be):
    perm, sorted_tokens, valid = route_and_pack(cfg, te)
    x16 = np.ascontiguousarray(xs).astype(np.float16)
    xT = np.ascontiguousarray(xs.T).astype(np.float16)          # [d, n_loc]
    xgT = np.ascontiguousarray(x16[sorted_tokens].T)            # [d, rt*128]
    WsT = np.ascontiguousarray(Ws.T).astype(np.float16)         # [d, h]
    # routed weights pre-tiled per (slot, n): [e*ns, 128, kc*512] with
    # [s*ns+n, p, k*512+j] = We[perm[s]].T[k*128+p, n*512+j]
    WeT = We[perm].transpose(0, 2, 1).astype(np.float16)        # [e, d, h]
    wet = (
        WeT.reshape(cfg.e, cfg.kc, 128, cfg.ns, 512)
        .transpose(0, 3, 2, 1, 4)
        .reshape(cfg.e * cfg.ns, 128, cfg.kc * 512)
    )
    return {
        "xt16": _part_tile(xT, cfg.kc),
        "xg16": _part_tile(xgT, cfg.kc),
        "wst16": _part_tile(WsT, cfg.kc),
        "wet16": np.ascontiguousarray(wet),
        "bs_rep": np.ascontiguousarray(
            np.broadcast_to(bs.astype(np.float32), (128, cfg.h))
        ),
        "be_rep": np.ascontiguousarray(
            np.broadcast_to(
                be[perm].astype(np.float32)[:, None, :], (cfg.e, 128, cfg.h)
            )
        ),
    }, (sorted_tokens, valid)


def combine(cfg: Cfg, shared_out, routed_out, sorted_tokens, valid):
    out = shared_out.astype(np.float32)
    out[sorted_tokens[valid]] += routed_out[valid].astype(np.float32)
    return out


# ----------------------------------------------------------------------------
# entry point
# ----------------------------------------------------------------------------

_PROGRAM_CACHE = {}


def _get_program(cfg: Cfg):
    key = (cfg.n_loc, cfg.d, cfg.h, cfg.e, cfg.cap)
    if key not in _PROGRAM_CACHE:
        _PROGRAM_CACHE[key] = build_program(cfg)
    return _PROGRAM_CACHE[key]


def kernel(x, Ws, bs, We, be, Wr, br):
    from concourse.bass_utils import run_bass_kernel_spmd

    cfg = Cfg()
    x = np.asarray(x, dtype=np.float32)
    Ws = np.asarray(Ws, dtype=np.float32)
    bs = np.asarray(bs, dtype=np.float32)
    We = np.asarray(We, dtype=np.float32)
    be = np.asarray(be, dtype=np.float32)
    Wr = np.asarray(Wr, dtype=np.float32)
    br = np.asarray(br, dtype=np.float32)

    n = x.shape[0]
    assert n == cfg.n_loc * cfg.n_cores

    logits = x @ Wr.T + br
    te = np.argmax(logits, axis=-1)

    nc = _get_program(cfg)
    in_maps, metas = [], []
    for c in range(cfg.n_cores):
        sl = slice(c * cfg.n_loc, (c + 1) * cfg.n_loc)
        m, meta = make_in_map(cfg, x[sl], te[sl], Ws, bs, We, be)
        in_maps.append(m)
        metas.append(meta)

    res = run_bass_kernel_spmd(nc, in_maps, list(range(cfg.n_cores)))
    outs = []
    for c in range(cfg.n_cores):
        st, valid = metas[c]
        outs.append(
            combine(cfg, res.results[c]["out"], res.results[c]["routed"], st, valid)
        )
    return np.concatenate(outs, axis=0)


# revision 9
# speedup vs baseline: 1.2043x; 1.0642x over previous
"""MoE (top-1 routing, E=8) Trainium2 Bass kernel.

Full-input contract: kernel(**inputs) takes the unsharded numpy inputs of
reference.setup_inputs() and returns the full [N, H] float32 output.

Strategy (token-parallel SPMD over 8 NeuronCores, 2048 tokens/core):
  host:   fp32 router (x @ Wr.T + br, argmax); per-core expert-sorted
          "capacity layout" (per-slot 128-aligned segments, experts
          permuted per core by descending count so one static capacity
          profile fits every core); tokens pre-gathered into the sorted
          layout; fp16 casts; weights/activations pre-tiled so every
          device DMA is one large fully-contiguous transfer with the
          contraction dim D on SBUF partitions.
  device: shared FFN = dense fp16 matmuls over token-order tiles
          (tokens stationary, H moving in 512-slices, K=8x128 chunks
          accumulated in PSUM fp32) + bs -> fp16 "out" rows.
          routed FFN = fp16 matmuls over the sorted capacity tiles with
          each slot's expert weights + be -> fp16 "routed" rows (dense,
          sorted layout).
  host:   out[token] = shared[token] + routed[slot_of(token)], fp32.
"""

import sys

sys.path.insert(0, "/opt/trn_rl_repo")

from dataclasses import dataclass

import numpy as np

import concourse.bass as bass
import concourse.mybir as mybir
from concourse.tile import TileContext

# ----------------------------------------------------------------------------
# configuration
# ----------------------------------------------------------------------------


@dataclass
class Cfg:
    n_loc: int = 2048          # tokens per core
    d: int = 1024              # input dim (contraction)
    h: int = 4096              # hidden dim
    e: int = 8                 # experts
    cap: tuple = (3, 3, 3, 3, 3, 2, 2, 2)  # tiles per sorted slot
    n_cores: int = 8

    @property
    def kc(self):
        return self.d // 128

    @property
    def ns(self):
        return self.h // 512

    @property
    def nt(self):
        return self.n_loc // 128

    @property
    def rt(self):
        return sum(self.cap)


F16 = mybir.dt.float16
F32 = mybir.dt.float32

MAX_WAITS = 1


def split_long_waits(nc, max_w: int = MAX_WAITS):
    """walrus TPB_CTRL codegen rejects instructions with multiple sync waits
    (CoreV3GenImpl setupSyncWait).  Tile's exit drain can exceed that; move
    excess waits onto same-engine NoOps inserted just before the offender."""
    n_fix = 0
    for f in nc.m.functions:
        for bb in f.blocks:
            insts = bb.instructions
            new_list = []
            changed = False
            for inst in insts:
                si = inst.sync_info
                if si is not None and len(si.on_wait) > max_w:
                    w = list(si.on_wait)
                    k = 0
                    while len(w) > max_w:
                        chunk, w = w[:max_w], w[max_w:]
                        nop = mybir.InstNoOp(
                            name=f"{inst.name}_waitsplit_{k}",
                            engine=inst.engine,
                            sync_info=mybir.SyncInfo(on_wait=chunk, on_update=[]),
                            bass_nofuse=True,
                        )
                        new_list.append(nop)
                        k += 1
                    inst.sync_info = mybir.SyncInfo(
                        on_wait=w, on_update=list(si.on_update)
                    )
                    n_fix += 1
                    changed = True
                new_list.append(inst)
            if changed:
                bb.instructions = new_list
    return n_fix


# ----------------------------------------------------------------------------
# device program
# ----------------------------------------------------------------------------


def build_program(cfg: Cfg, fix_waits: bool = True):
    nc = bass.Bass()

    # all activation/weight params are pre-tiled on the host so that each
    # DMA below is a single fully-contiguous transfer.
    xt = nc.declare_dram_parameter(
        "xt16", [128, cfg.kc * cfg.n_loc], F16, isOutput=False
    )
    xg_d = nc.declare_dram_parameter(
        "xg16", [128, cfg.kc * cfg.rt * 128], F16, isOutput=False
    )
    wst = nc.declare_dram_parameter(
        "wst16", [128, cfg.kc * cfg.h], F16, isOutput=False
    )
    wet = nc.declare_dram_parameter(
        "wet16", [cfg.e * cfg.ns, 128, cfg.kc * 512], F16, isOutput=False
    )
    bsr = nc.declare_dram_parameter("bs_rep", [128, cfg.h], F16, isOutput=False)
    ber = nc.declare_dram_parameter(
        "be_rep", [cfg.e, 128, cfg.h], F32, isOutput=False
    )
    outp = nc.declare_dram_parameter("out", [cfg.n_loc, cfg.h], F16, isOutput=True)
    routp = nc.declare_dram_parameter(
        "routed", [cfg.rt * 128, cfg.h], F16, isOutput=True
    )

    base = np.cumsum([0] + list(cfg.cap))  # slot -> first tile index
    cap_max = max(cfg.cap)

    with TileContext(nc) as tc:
        with (
            tc.tile_pool(name="resident", bufs=1) as rpool,
            tc.tile_pool(name="wrt", bufs=3) as wepool,
            tc.tile_pool(name="ps", bufs=6, space="PSUM") as pspool,
        ):
            # ---- resident loads (each one contiguous DMA) -----------------
            xg = rpool.tile([128, cfg.kc, cfg.rt * 128], F16, tag="xg")
            nc.sync.dma_start(out=xg[:, :, :], in_=xg_d[:, :])

            # ---- shared FFN: dense token-order tiles, full-row stores -----
            with (
                tc.tile_pool(name="sh_res", bufs=1) as spool,
                tc.tile_pool(name="oshared", bufs=2) as opool,
            ):
                xts = spool.tile([128, cfg.kc, cfg.n_loc], F16, tag="xts")
                wsall = spool.tile([128, cfg.kc, cfg.h], F16, tag="ws")
                for k in range(cfg.kc):
                    nc.sync.dma_start(
                        out=xts[:, k, :],
                        in_=xt[:, k * cfg.n_loc : (k + 1) * cfg.n_loc],
                    )
                    nc.sync.dma_start(
                        out=wsall[:, k, :],
                        in_=wst[:, k * cfg.h : (k + 1) * cfg.h],
                    )
                bs_sb = spool.tile([128, cfg.h], F16, tag="bs")
                nc.sync.dma_start(out=bs_sb[:, :], in_=bsr[:, :])

                for t in range(cfg.nt):
                    sh = opool.tile([128, cfg.h], F16, tag="osh")
                    for n in range(cfg.ns):
                        ps = pspool.tile([128, 512], F32, tag="ps")
                        for k in range(cfg.kc):
                            nc.tensor.matmul(
                                ps[:, :],
                                lhsT=xts[:, k, t * 128 : (t + 1) * 128],
                                rhs=wsall[:, k, n * 512 : (n + 1) * 512],
                                start=(k == 0),
                                stop=(k == cfg.kc - 1),
                            )
                        nc.vector.tensor_add(
                            out=sh[:, n * 512 : (n + 1) * 512],
                            in0=ps[:, :],
                            in1=bs_sb[:, n * 512 : (n + 1) * 512],
                        )
                    nc.sync.dma_start(
                        out=outp[t * 128 : (t + 1) * 128, :], in_=sh[:, :]
                    )

            # ---- routed FFN: sorted capacity layout -----------------------
            with (
                tc.tile_pool(name="stage", bufs=2) as stpool,
                tc.tile_pool(name="bias_e", bufs=2) as bpool,
            ):
                for s in range(cfg.e):
                    bes = bpool.tile([128, cfg.h], F32, tag="be")
                    nc.sync.dma_start(out=bes[:, :], in_=ber[s, :, :])
                    st = stpool.tile([128, cap_max, cfg.h], F16, tag="st")
                    for n in range(cfg.ns):
                        wtile = wepool.tile([128, cfg.kc, 512], F16, tag="we")
                        nc.sync.dma_start(
                            out=wtile[:, :, :], in_=wet[s * cfg.ns + n, :, :]
                        )
                        for tl in range(cfg.cap[s]):
                            t = base[s] + tl
                            ps = pspool.tile([128, 512], F32, tag="ps")
                            for k in range(cfg.kc):
                                nc.tensor.matmul(
                                    ps[:, :],
                                    lhsT=xg[:, k, t * 128 : (t + 1) * 128],
                                    rhs=wtile[:, k, :],
                                    start=(k == 0),
                                    stop=(k == cfg.kc - 1),
                                )
                            nc.vector.tensor_add(
                                out=st[:, tl, n * 512 : (n + 1) * 512],
                                in0=ps[:, :],
                                in1=bes[:, n * 512 : (n + 1) * 512],
                            )
                    for tl in range(cfg.cap[s]):
                        t = base[s] + tl
                        nc.sync.dma_start(
                            out=routp[t * 128 : (t + 1) * 128, :],
                            in_=st[:, tl, :],
                        )

    if fix_waits:
        split_long_waits(nc)
    return nc


# ----------------------------------------------------------------------------
# host-side routing / input prep / combine
# ----------------------------------------------------------------------------


def _part_tile(a, kc):
    """[kc*128, F] -> [128, kc*F] with [p, k*F+j] = a[k*128+p, j]."""
    f = a.shape[1]
    return a.reshape(kc, 128, f).transpose(1, 0, 2).reshape(128, kc * f)


def route_and_pack(cfg: Cfg, te):
    """Per-core routing tables.  te [n_loc] expert ids.

    Returns (perm, sorted_tokens, valid): sorted_tokens [rt*128] maps
    capacity slot -> token id (pad slots -> token 0, valid False)."""
    counts = np.bincount(te, minlength=cfg.e)
    perm = np.argsort(-counts, kind="stable")
    base = np.cumsum([0] + list(cfg.cap))
    sorted_tokens = np.zeros(cfg.rt * 128, dtype=np.int64)
    valid = np.zeros(cfg.rt * 128, dtype=bool)
    for s in range(cfg.e):
        ex = perm[s]
        toks = np.nonzero(te == ex)[0]
        assert len(toks) <= cfg.cap[s] * 128, (
            f"slot {s} expert {ex}: {len(toks)} tokens > capacity "
            f"{cfg.cap[s] * 128}"
        )
        off = base[s] * 128
        sorted_tokens[off : off + len(toks)] = toks
        valid[off : off + len(toks)] = True
    return perm, sorted_tokens, valid


def make_in_map(cfg: Cfg, xs, te, Ws, bs, We, be):
    perm, sorted_tokens, valid = route_and_pack(cfg, te)
    x16 = np.ascontiguousarray(xs).astype(np.float16)
    xT = np.ascontiguousarray(xs.T).astype(np.float16)          # [d, n_loc]
    xgT = np.ascontiguousarray(x16[sorted_tokens].T)            # [d, rt*128]
    WsT = np.ascontiguousarray(Ws.T).astype(np.float16)         # [d, h]
    # routed weights pre-tiled per (slot, n): [e*ns, 128, kc*512] with
    # [s*ns+n, p, k*512+j] = We[perm[s]].T[k*128+p, n*512+j]
    WeT = We[perm].transpose(0, 2, 1).astype(np.float16)        # [e, d, h]
    wet = (
        WeT.reshape(cfg.e, cfg.kc, 128, cfg.ns, 512)
        .transpose(0, 3, 2, 1, 4)
        .reshape(cfg.e * cfg.ns, 128, cfg.kc * 512)
    )
    return {
        "xt16": _part_tile(xT, cfg.kc),
        "xg16": _part_tile(xgT, cfg.kc),
        "wst16": _part_tile(WsT, cfg.kc),
        "wet16": np.ascontiguousarray(wet),
        "bs_rep": np.ascontiguousarray(
            np.broadcast_to(bs.astype(np.float16), (128, cfg.h))
        ),
        "be_rep": np.ascontiguousarray(
            np.broadcast_to(
                be[perm].astype(np.float32)[:, None, :], (cfg.e, 128, cfg.h)
            )
        ),
    }, (sorted_tokens, valid)


def combine(cfg: Cfg, shared_out, routed_out, sorted_tokens, valid):
    out = shared_out.astype(np.float32)
    out[sorted_tokens[valid]] += routed_out[valid].astype(np.float32)
    return out


# ----------------------------------------------------------------------------
# entry point
# ----------------------------------------------------------------------------

_PROGRAM_CACHE = {}


def _get_program(cfg: Cfg):
    key = (cfg.n_loc, cfg.d, cfg.h, cfg.e, cfg.cap)
    if key not in _PROGRAM_CACHE:
        _PROGRAM_CACHE[key] = build_program(cfg)
    return _PROGRAM_CACHE[key]


def kernel(x, Ws, bs, We, be, Wr, br):
    from concourse.bass_utils import run_bass_kernel_spmd

    cfg = Cfg()
    x = np.asarray(x, dtype=np.float32)
    Ws = np.asarray(Ws, dtype=np.float32)
    bs = np.asarray(bs, dtype=np.float32)
    We = np.asarray(We, dtype=np.float32)
    be = np.asarray(be, dtype=np.float32)
    Wr = np.asarray(Wr, dtype=np.float32)
    br = np.asarray(br, dtype=np.float32)

    n = x.shape[0]
    assert n == cfg.n_loc * cfg.n_cores

    logits = x @ Wr.T + br
    te = np.argmax(logits, axis=-1)

    nc = _get_program(cfg)
    in_maps, metas = [], []
    for c in range(cfg.n_cores):
        sl = slice(c * cfg.n_loc, (c + 1) * cfg.n_loc)
        m, meta = make_in_map(cfg, x[sl], te[sl], Ws, bs, We, be)
        in_maps.append(m)
        metas.append(meta)

    res = run_bass_kernel_spmd(nc, in_maps, list(range(cfg.n_cores)))
    outs = []
    for c in range(cfg.n_cores):
        st, valid = metas[c]
        outs.append(
            combine(cfg, res.results[c]["out"], res.results[c]["routed"], st, valid)
        )
    return np.concatenate(outs, axis=0)
